# revision 1
# baseline (speedup 1.0000x reference)
"""Trainium2 Bass kernel for single-query attention over per-sample concepts.

    sab[b, k] = (query[b] . concept[b, k]) / sqrt(D)
    score     = softmax(sab, axis=-1)
    out[b]    = sum_k score[b, k] * concept[b, k]

Shapes: query [256, 1024] f32, concept [256, 2048, 1024] f32 -> out [256, 1024].

Sharding: pure data parallel, batch 256 split as 32 samples on each of 8
NeuronCores. Memory-bound: each core streams its 256 MiB concept shard once.

Per-core dataflow, per sample b (all tiles [128 k-partitions, 1024 d-free]):
  - DMA qb = broadcast(query[b]) to 128 partitions        (ACT HWDGE ring)
  - DMA c-tile t (128 k's)                                 (SP HWDGE ring)
  - DVE scalar_tensor_tensor: elementwise (c*scale)*qb with accum_out
    -> raw scores s[128, 1] per tile (fused multiply+reduce, one pass)
  - ACT exp per tile -> e[128, 1]
  - PE matmul: acc[1, 0:512] += e_t.T @ c_t[:, 0:512], same for 512:1024
    (fp32, PSUM accumulate over the 16 k-tiles)
  - denominator: ACT copy of e-columns with accum_out -> per-partition sums,
    then PE matmul with ones stationary -> [1, 1] in PSUM
  - DVE reciprocal, ACT Copy-with-scale to normalize, DMA out row.
"""

import numpy as np
from contextlib import ExitStack

import concourse.bacc as bacc
import concourse.tile as tile
from concourse import mybir
from concourse.bass_utils import run_bass_kernel_spmd

B, K, D = 256, 2048, 1024
NCORES = 8
BL = B // NCORES          # 32 samples per core
KT = 128                  # k-tile size (partition dim)
NT = K // KT              # 16 k-tiles per sample
SCALE = 1.0 / float(np.sqrt(D))

_cache = {}


def build_nc():
    nc = bacc.Bacc("TRN2", target_bir_lowering=False, debug=False,
                   num_devices=NCORES)
    q = nc.dram_tensor("query", [BL, D], mybir.dt.float32, kind="ExternalInput")
    c = nc.dram_tensor("concept", [BL, K, D], mybir.dt.float32,
                       kind="ExternalInput")
    out = nc.dram_tensor("out", [BL, D], mybir.dt.float32,
                         kind="ExternalOutput")
    f32 = mybir.dt.float32

    with tile.TileContext(nc) as tc, ExitStack() as ctx:
        cpool = ctx.enter_context(tc.tile_pool(name="c", bufs=10))
        qpool = ctx.enter_context(tc.tile_pool(name="q", bufs=3))
        spool = ctx.enter_context(tc.tile_pool(name="scr", bufs=2))
        epool = ctx.enter_context(tc.tile_pool(name="e", bufs=3))
        onepool = ctx.enter_context(tc.tile_pool(name="one", bufs=1))
        opool = ctx.enter_context(tc.tile_pool(name="o", bufs=4))
        ppool = ctx.enter_context(tc.tile_pool(name="ps", bufs=2, space="PSUM"))
        dpool = ctx.enter_context(tc.tile_pool(name="dn", bufs=2, space="PSUM"))

        ones = onepool.tile([KT, 1], f32)
        nc.vector.memset(ones[:], 1.0)

        for b in range(BL):
            qb = qpool.tile([KT, D], f32)
            nc.scalar.dma_start(out=qb[:], in_=q[b : b + 1, :].to_broadcast((KT, D)))

            scols = epool.tile([KT, NT], f32)
            ecols = epool.tile([KT, NT], f32)
            acc_lo = ppool.tile([1, 512], f32)
            acc_hi = ppool.tile([1, 512], f32)

            for t in range(NT):
                ct = cpool.tile([KT, D], f32)
                nc.sync.dma_start(out=ct[:], in_=c[b, t * KT : (t + 1) * KT, :])
                scr = spool.tile([KT, D], f32)
                nc.vector.scalar_tensor_tensor(
                    out=scr[:],
                    in0=ct[:],
                    scalar=SCALE,
                    in1=qb[:],
                    op0=mybir.AluOpType.mult,
                    op1=mybir.AluOpType.mult,
                    accum_out=scols[:, t : t + 1],
                )
                nc.scalar.activation(
                    out=ecols[:, t : t + 1],
                    in_=scols[:, t : t + 1],
                    func=mybir.ActivationFunctionType.Exp,
                )
                e_t = ecols[:, t : t + 1]
                nc.tensor.matmul(acc_lo[:], e_t, ct[:, 0:512],
                                 start=(t == 0), stop=(t == NT - 1))
                nc.tensor.matmul(acc_hi[:], e_t, ct[:, 512:1024],
                                 start=(t == 0), stop=(t == NT - 1))

            # denominator: per-partition sums of e, then reduce across
            # partitions with a ones-stationary matmul
            ered = epool.tile([KT, 1], f32)
            escr = spool.tile([KT, NT], f32)
            nc.scalar.activation(
                out=escr[:],
                in_=ecols[:],
                func=mybir.ActivationFunctionType.Copy,
                accum_out=ered[:],
            )
            denom = dpool.tile([1, 1], f32)
            nc.tensor.matmul(denom[:], ones[:], ered[:], start=True, stop=True)

            recip = opool.tile([1, 1], f32)
            nc.vector.reciprocal(recip[:], denom[:])

            orow = opool.tile([1, D], f32)
            nc.scalar.activation(out=orow[:, 0:512], in_=acc_lo[:],
                                 func=mybir.ActivationFunctionType.Copy,
                                 scale=recip[:])
            nc.scalar.activation(out=orow[:, 512:1024], in_=acc_hi[:],
                                 func=mybir.ActivationFunctionType.Copy,
                                 scale=recip[:])
            nc.scalar.dma_start(out=out[b : b + 1, :], in_=orow[:])

    nc.compile()
    return nc


def _run(query, concept, trace=False, trace_kwargs=None):
    if "nc" not in _cache:
        _cache["nc"] = build_nc()
    nc = _cache["nc"]
    in_maps = []
    for i in range(NCORES):
        in_maps.append({
            "query": np.ascontiguousarray(query[i * BL : (i + 1) * BL]),
            "concept": np.ascontiguousarray(concept[i * BL : (i + 1) * BL]),
        })
    res = run_bass_kernel_spmd(
        nc, in_maps, core_ids=list(range(NCORES)),
        trace=trace, **(trace_kwargs or {}),
    )
    out = np.concatenate([res.results[i]["out"] for i in range(NCORES)], axis=0)
    return out.astype(np.float32), res


def kernel(query: np.ndarray, concept: np.ndarray) -> np.ndarray:
    out, _ = _run(np.asarray(query, np.float32), np.asarray(concept, np.float32))
    return out


# revision 5
# speedup vs baseline: 1.2047x; 1.2047x over previous
"""Trainium2 Bass kernel for single-query attention over per-sample concepts.

    sab[b, k] = (query[b] . concept[b, k]) / sqrt(D)
    score     = softmax(sab, axis=-1)
    out[b]    = sum_k score[b, k] * concept[b, k]

Shapes: query [256, 1024] f32, concept [256, 2048, 1024] f32 -> out [256, 1024].

Sharding: pure data parallel, batch 256 split as 32 samples on each of 8
NeuronCores. Memory-bound: each core streams its 256 MiB concept shard once.

Per-core dataflow, per sample b (all tiles [128 k-partitions, 1024 d-free]):
  - DMA qb = broadcast(query[b]) to 128 partitions        (ACT HWDGE ring)
  - DMA c-tile t (128 k's)                                 (SP HWDGE ring)
  - DVE scalar_tensor_tensor: elementwise (c*scale)*qb with accum_out
    -> raw scores s[128, 1] per tile (fused multiply+reduce, one pass)
  - ACT exp per tile -> e[128, 1]
  - PE matmul: acc[1, 0:512] += e_t.T @ c_t[:, 0:512], same for 512:1024
    (fp32, PSUM accumulate over the 16 k-tiles)
  - denominator: ACT copy of e-columns with accum_out -> per-partition sums,
    then PE matmul with ones stationary -> [1, 1] in PSUM
  - DVE reciprocal, ACT Copy-with-scale to normalize, DMA out row.
"""

import numpy as np
from contextlib import ExitStack

import concourse.bacc as bacc
import concourse.tile as tile
from concourse import mybir
from concourse.bass_utils import run_bass_kernel_spmd

B, K, D = 256, 2048, 1024
NCORES = 8
BL = B // NCORES          # 32 samples per core
KT = 128                  # k-tile size (partition dim)
NT = K // KT              # 16 k-tiles per sample
SCALE = 1.0 / float(np.sqrt(D))

_cache = {}


def build_nc():
    nc = bacc.Bacc("TRN2", target_bir_lowering=False, debug=False,
                   num_devices=NCORES)
    q = nc.dram_tensor("query", [BL, D], mybir.dt.float32, kind="ExternalInput")
    c = nc.dram_tensor("concept", [BL, K, D], mybir.dt.float32r,
                       kind="ExternalInput")
    out = nc.dram_tensor("out", [BL, D], mybir.dt.float32,
                         kind="ExternalOutput")
    f32 = mybir.dt.float32

    f32r = mybir.dt.float32r

    with tile.TileContext(nc) as tc, ExitStack() as ctx:
        cpool = ctx.enter_context(tc.tile_pool(name="c", bufs=16))
        qpool = ctx.enter_context(tc.tile_pool(name="q", bufs=3))
        spool = ctx.enter_context(tc.tile_pool(name="scr", bufs=2))
        epool = ctx.enter_context(tc.tile_pool(name="e", bufs=3))
        onepool = ctx.enter_context(tc.tile_pool(name="one", bufs=1))
        opool = ctx.enter_context(tc.tile_pool(name="o", bufs=4))
        ppool = ctx.enter_context(tc.tile_pool(name="ps", bufs=2, space="PSUM"))
        dpool = ctx.enter_context(tc.tile_pool(name="dn", bufs=2, space="PSUM"))

        ones = onepool.tile([KT, 1], f32)
        nc.vector.memset(ones[:], 1.0)

        for b in range(BL):
            qrow = qpool.tile([1, D], f32)
            nc.scalar.dma_start(out=qrow[:], in_=q[b : b + 1, :])
            qb = qpool.tile([KT, D], f32)
            nc.gpsimd.partition_broadcast(qb[:], qrow[:])

            scols = epool.tile([KT, NT], f32)
            ecols = epool.tile([KT, NT], f32r)
            acc_lo = ppool.tile([1, 512], f32)
            acc_hi = ppool.tile([1, 512], f32)

            for t in range(NT):
                ct = cpool.tile([KT, D], f32r)
                nc.sync.dma_start(out=ct[:], in_=c[b, t * KT : (t + 1) * KT, :])
                scr = spool.tile([KT, D], f32)
                nc.vector.scalar_tensor_tensor(
                    out=scr[:],
                    in0=ct[:].bitcast(f32),
                    scalar=SCALE,
                    in1=qb[:],
                    op0=mybir.AluOpType.mult,
                    op1=mybir.AluOpType.mult,
                    accum_out=scols[:, t : t + 1],
                )
                nc.scalar.activation(
                    out=ecols[:, t : t + 1],
                    in_=scols[:, t : t + 1],
                    func=mybir.ActivationFunctionType.Exp,
                )
                e_t = ecols[:, t : t + 1]
                nc.tensor.matmul(acc_lo[:], e_t, ct[:, 0:512],
                                 start=(t == 0), stop=(t == NT - 1))
                nc.tensor.matmul(acc_hi[:], e_t, ct[:, 512:1024],
                                 start=(t == 0), stop=(t == NT - 1))

            # denominator: per-partition sums of e, then reduce across
            # partitions with a ones-stationary matmul
            ered = epool.tile([KT, 1], f32)
            escr = spool.tile([KT, NT], f32)
            nc.scalar.activation(
                out=escr[:],
                in_=ecols[:].bitcast(f32),
                func=mybir.ActivationFunctionType.Copy,
                accum_out=ered[:],
            )
            denom = dpool.tile([1, 1], f32)
            nc.tensor.matmul(denom[:], ones[:], ered[:], start=True, stop=True)

            recip = opool.tile([1, 1], f32)
            nc.vector.reciprocal(recip[:], denom[:])

            orow = opool.tile([1, D], f32)
            nc.scalar.activation(out=orow[:, 0:512], in_=acc_lo[:],
                                 func=mybir.ActivationFunctionType.Copy,
                                 scale=recip[:])
            nc.scalar.activation(out=orow[:, 512:1024], in_=acc_hi[:],
                                 func=mybir.ActivationFunctionType.Copy,
                                 scale=recip[:])
            nc.scalar.dma_start(out=out[b : b + 1, :], in_=orow[:])

    nc.compile()
    return nc


def _run(query, concept, trace=False, trace_kwargs=None):
    if "nc" not in _cache:
        _cache["nc"] = build_nc()
    nc = _cache["nc"]
    in_maps = []
    for i in range(NCORES):
        in_maps.append({
            "query": np.ascontiguousarray(query[i * BL : (i + 1) * BL]),
            "concept": np.ascontiguousarray(concept[i * BL : (i + 1) * BL]),
        })
    res = run_bass_kernel_spmd(
        nc, in_maps, core_ids=list(range(NCORES)),
        trace=trace, **(trace_kwargs or {}),
    )
    out = np.concatenate([res.results[i]["out"] for i in range(NCORES)], axis=0)
    return out.astype(np.float32), res


def kernel(query: np.ndarray, concept: np.ndarray) -> np.ndarray:
    out, _ = _run(np.asarray(query, np.float32), np.asarray(concept, np.float32))
    return out
